# revision 21
# baseline (speedup 1.0000x reference)
"""Trainium2 Bass kernel for nn_Atten_28535762715239.

Reference computation (B=4, S=2048, HID=1024), per batch b:
    Q = relu(q[b] @ wQ.T + bq)        # [S, D]
    K = relu(k[b] @ wK.T + bk)        # [S, D]
    V = relu(v[b] @ wQ.T + bq)        # NOTE: reference reuses wQ for V
    S_ = (Q @ K.T) / sqrt(D)          # [S, S]
    attn = softmax(S_, -1)            # output 2
    out  = (attn @ V) @ wO.T + bo     # output 1

Sharding: 8 cores = 4 batches x 2 query-halves. Core cid=2*b+h computes
queries [h*1024,(h+1)*1024) of batch b. K/V projections are computed
redundantly by the 2 cores of a batch (cheaper than cross-core collectives).

On-chip dataflow (matmul operands bf16, fp32 PSUM accumulation):
    DMA xbar-transposed loads give x^T chunks [128d, seq]
    Q^T[e,i], K^T[e,j] = (w^T chunk).T @ x^T chunk, DVE evict bias+relu
    V[j,e] = (xv^T chunk).T @ wqT, bias via K=1 ones-matmul, DVE relu
    S tile = (Q^T).T @ K^T -> PSUM -> ACT exp(S/32) -> E fp32 + row sums
    attn = E * (1/sum)   (DVE per-partition scalar) -> gpsimd DMA out
    E^T via DMA transpose (bf16); ctx = (E^T).T @ V, scaled by 1/sum
    ctx^T via DMA transpose; out = (ctx^T).T @ woT + bo -> gpsimd DMA out

Engine split: PE matmuls; ACT only exp (single table); DVE all other
psum evictions + softmax arithmetic; SP(HWDGE) loads + transposes;
Pool(SWDGE) output stores. This keeps the latency-critical E^T/ctx^T
transposes from queuing behind stores on the SP sequencer.
"""

import os
import sys

for _p in ("/opt/trn_rl_repo", "/opt/pypackages"):
    if _p not in sys.path:
        sys.path.append(_p)

import numpy as np
import ml_dtypes

import concourse.bass as bass
import concourse.tile as tile
from concourse import mybir
from concourse.bass_utils import run_bass_kernel_spmd

BF16 = mybir.dt.bfloat16
F32 = mybir.dt.float32
AF = mybir.ActivationFunctionType
AX = mybir.AxisListType
OP = mybir.AluOpType
bf16 = ml_dtypes.bfloat16

B, S, D = 4, 2048, 1024
SQ = 1024          # queries per core
N_CORES = 8
NDC = D // 128     # 8 contraction chunks
NT = SQ // 128     # 8 query tiles per core
NJR = S // 512     # 4 key ranges
NJC = S // 128     # 16 key chunks
SCALE = 1.0 / 32.0  # 1/sqrt(D)


def _split_excess_waits(nc, max_w=1):
    """walrus CTRL codegen rejects instructions carrying more than one sem
    wait; spill extras onto preceding same-engine nops."""
    n = 0
    for f in nc.m.functions:
        for b in f.blocks:
            out = []
            for inst in b.instructions:
                si = inst.sync_info
                if si is not None and si.on_wait and len(si.on_wait) > max_w:
                    waits = list(si.on_wait)
                    extra, keep = waits[:-max_w], waits[-max_w:]
                    for j in range(0, len(extra), max_w):
                        nop = mybir.InstNoOp(
                            name=f"WSPLIT-{n}",
                            engine=inst.engine,
                            sync_info=mybir.SyncInfo(
                                on_wait=extra[j : j + max_w], on_update=[]
                            ),
                            bass_nofuse=True,
                        )
                        n += 1
                        out.append(nop)
                    si.on_wait = keep
                    inst.sync_info = si
                out.append(inst)
            b.instructions[:] = out
    return n


def build_program(split_waits=True):
    nc = bass.Bass(
        "TRN2", target_bir_lowering=False, debug=False, num_devices=N_CORES
    )

    def inp(name, shape, dt):
        return nc.dram_tensor(name, shape, dt, kind="ExternalInput").ap()

    def outp(name, shape, dt):
        return nc.dram_tensor(name, shape, dt, kind="ExternalOutput").ap()

    xq = inp("xq", [D, SQ], BF16)   # q[b,half].T (host-pretransposed)
    xk = inp("xk", [D, S], BF16)    # k[b].T
    xv = inp("xv", [D, S], BF16)    # v[b].T
    wqT = inp("wqT", [D, D], BF16)
    wkT = inp("wkT", [D, D], BF16)
    woT = inp("woT", [D, D], BF16)
    bq_col = inp("bq_col", [128, NDC], F32)
    bk_col = inp("bk_col", [128, NDC], F32)
    bq_row = inp("bq_row", [1, D], BF16)
    bo_row = inp("bo_row", [1, D], BF16)
    attn_o = outp("attn_o", [SQ, S], F32)
    out_o = outp("out_o", [SQ, D], F32)

    with tile.TileContext(nc) as tc:
        from contextlib import ExitStack

        with ExitStack() as ctx:
            psum = ctx.enter_context(tc.tile_pool(name="psum", bufs=8, space="PSUM"))
            consts = ctx.enter_context(tc.tile_pool(name="consts", bufs=1))
            wpool = ctx.enter_context(tc.tile_pool(name="w", bufs=2))
            xin = ctx.enter_context(tc.tile_pool(name="xin", bufs=12))
            qt_p = ctx.enter_context(tc.tile_pool(name="qt", bufs=1))
            kt_p = ctx.enter_context(tc.tile_pool(name="kt", bufs=1))
            v_p = ctx.enter_context(tc.tile_pool(name="v", bufs=1))
            ef_p = ctx.enter_context(tc.tile_pool(name="ef", bufs=3))
            ebf_p = ctx.enter_context(tc.tile_pool(name="ebf", bufs=2))
            et_p = ctx.enter_context(tc.tile_pool(name="et", bufs=3))
            cx_p = ctx.enter_context(tc.tile_pool(name="cx", bufs=2))
            cxt_p = ctx.enter_context(tc.tile_pool(name="cxt", bufs=2))
            ob_p = ctx.enter_context(tc.tile_pool(name="ob", bufs=2))
            st_p = ctx.enter_context(tc.tile_pool(name="st", bufs=4))

            with nc.named_scope("setup"):
                ones_col = consts.tile([1, 128], BF16)
                nc.vector.memset(ones_col, 1.0)
                # dummy matmuls during the initial DMA ramp: ratchet the PE
                # p-state up so the first real matmuls run at full clock
                warm = consts.tile([1, 256], BF16)
                nc.vector.memset(warm, 1.0)
                ident = consts.tile([128, 128], BF16)
                from concourse.masks import make_identity
                make_identity(nc, ident)
                warm_ps = psum.tile([128, 512], F32, tag="ps", name="warm_ps")
                for _ in range(12):
                    nc.tensor.matmul(
                        warm_ps[:, :256], lhsT=warm[:, :128], rhs=warm,
                        start=True, stop=True,
                    )
                bqc = consts.tile([128, NDC], F32)
                nc.gpsimd.dma_start(out=bqc, in_=bq_col)
                bkc = consts.tile([128, NDC], F32)
                nc.gpsimd.dma_start(out=bkc, in_=bk_col)
                bqr = consts.tile([1, D], BF16)
                nc.gpsimd.dma_start(out=bqr, in_=bq_row)
                bor = consts.tile([1, D], BF16)
                nc.gpsimd.dma_start(out=bor, in_=bo_row)

            QT = qt_p.tile([128, NDC, SQ], BF16)
            KT = kt_p.tile([128, NDC, S], BF16)
            V = v_p.tile([128, NJC, D], BF16)

            def load_chunks(src, r0, nrows):
                chunks = []
                for c in range(NDC):
                    ch = xin.tile([128, nrows], BF16, tag="xin")
                    nc.sync.dma_start(
                        out=ch, in_=src[c * 128 : (c + 1) * 128, r0 : r0 + nrows]
                    )
                    chunks.append(ch)
                return chunks

            def relu_bias_evict(dst, ps, bias_ap):
                # dst = max(ps + bias, 0) on DVE (per-partition f32 bias)
                nc.vector.tensor_scalar(
                    out=dst, in0=ps, scalar1=bias_ap, scalar2=0.0,
                    op0=OP.add, op1=OP.max,
                )

            # ---- Q projection: QT[e, i] ----
            with nc.named_scope("qproj"):
                with nc.named_scope("wload"):
                    wq_sb = wpool.tile([128, NDC, D], BF16, tag="w")
                    wk_sb = wpool.tile([128, NDC, D], BF16, tag="w")
                first_chunks = []
                for c in range(NDC):
                    ch = xin.tile([128, 512], BF16, tag="xin", name=f"xq0_{c}")
                    nc.sync.dma_start(
                        out=ch, in_=xq[c * 128 : (c + 1) * 128, 0:512]
                    )
                    first_chunks.append(ch)
                    nc.scalar.dma_start(
                        out=wq_sb[:, c, :], in_=wqT[c * 128 : (c + 1) * 128, :]
                    )
                with nc.named_scope("wload"):
                    for c in range(NDC):
                        nc.scalar.dma_start(
                            out=wk_sb[:, c, :], in_=wkT[c * 128 : (c + 1) * 128, :]
                        )
                for ir in range(SQ // 512):
                    chunks = first_chunks if ir == 0 else load_chunks(xq, ir * 512, 512)
                    pss = [psum.tile([128, 512], F32, tag="ps", name=f"ps_q{ir}_{et}")
                           for et in range(NDC)]
                    for c in range(NDC):
                        for et in range(NDC):
                            nc.tensor.matmul(
                                pss[et],
                                lhsT=wq_sb[:, c, et * 128 : (et + 1) * 128],
                                rhs=chunks[c],
                                start=(c == 0),
                                stop=(c == NDC - 1),
                            )
                    for et in range(NDC):
                        relu_bias_evict(
                            QT[:, et, ir * 512 : (ir + 1) * 512], pss[et],
                            bqc[:, et : et + 1],
                        )

            # ---- K projection: KT[e, j] ----
            with nc.named_scope("kproj"):
                for jr in range(NJR):
                    chunks = load_chunks(xk, jr * 512, 512)
                    pss = [psum.tile([128, 512], F32, tag="ps", name=f"ps_k{jr}_{et}")
                           for et in range(NDC)]
                    for c in range(NDC):
                        for et in range(NDC):
                            nc.tensor.matmul(
                                pss[et],
                                lhsT=wk_sb[:, c, et * 128 : (et + 1) * 128],
                                rhs=chunks[c],
                                start=(c == 0),
                                stop=(c == NDC - 1),
                            )
                    for et in range(NDC):
                        relu_bias_evict(
                            KT[:, et, jr * 512 : (jr + 1) * 512], pss[et],
                            bkc[:, et : et + 1],
                        )

            Ef, sums, recips, EBs, ETs, CXs, CXTs = {}, {}, {}, {}, {}, {}, {}

            def scores(t):
                # stationary QT[ec, i-tile] shared across the 4 j-ranges
                with nc.named_scope(f"scores{t}"):
                    e = ef_p.tile([128, S], F32, tag="ef")
                    eb = ebf_p.tile([128, S], BF16, tag="ebf")
                    sm = st_p.tile([128, NJR], F32, tag="sums")
                    pss = [psum.tile([128, 512], F32, tag="ps", name=f"ps_s{t}_{jr}") for jr in range(NJR)]
                    for ec in range(NDC):
                        for jr in range(NJR):
                            nc.tensor.matmul(
                                pss[jr],
                                lhsT=QT[:, ec, t * 128 : (t + 1) * 128],
                                rhs=KT[:, ec, jr * 512 : (jr + 1) * 512],
                                start=(ec == 0),
                                stop=(ec == NDC - 1),
                            )
                    for jr in range(NJR):
                        # bf16 copy of exp for the ctx matmul: produced
                        # straight from PSUM so the E^T transpose can start
                        # two iterations before ctx(t) consumes it
                        nc.scalar.activation(
                            out=eb[:, jr * 512 : (jr + 1) * 512],
                            in_=pss[jr],
                            func=AF.Exp,
                            scale=SCALE,
                        )
                        nc.scalar.activation(
                            out=e[:, jr * 512 : (jr + 1) * 512],
                            in_=pss[jr],
                            func=AF.Exp,
                            scale=SCALE,
                            accum_out=sm[:, jr : jr + 1],
                        )
                    EBs[t] = eb
                    Ef[t], sums[t] = e, sm

            def transpose_eb(t):
                # E^T via PE transpose (xbar transpose DMAs corrupt under
                # concurrent plain DMAs on this terminal)
                with nc.named_scope(f"teb{t}"):
                    eb = EBs[t]
                    ett = et_p.tile([128, NJC, 128], BF16, tag="et")
                    for jc in range(NJC):
                        tp = psum.tile([128, 128], BF16, tag="ps", name=f"tpe{t}_{jc}")
                        nc.tensor.transpose(
                            tp, eb[:, jc * 128 : (jc + 1) * 128], ident
                        )
                        nc.vector.tensor_copy(out=ett[:, jc, :], in_=tp)
                    ETs[t] = ett

            def transpose_cx(t):
                with nc.named_scope(f"tcx{t}"):
                    cx = CXs[t]
                    cxt = cxt_p.tile([128, NDC, 128], BF16, tag="cxt")
                    for ec in range(NDC):
                        tp = psum.tile([128, 128], BF16, tag="ps", name=f"tpc{t}_{ec}")
                        nc.tensor.transpose(
                            tp, cx[:, ec * 128 : (ec + 1) * 128], ident
                        )
                        nc.vector.tensor_copy(out=cxt[:, ec, :], in_=tp)
                    CXTs[t] = cxt

            def softmax_tail(t):
                with nc.named_scope(f"tail{t}"):
                    e, sm = Ef[t], sums[t]
                    tot = st_p.tile([128, 1], F32, tag="tot")
                    nc.vector.reduce_sum(out=tot, in_=sm, axis=AX.X)
                    rc = st_p.tile([128, 1], F32, tag="rc")
                    nc.vector.reciprocal(out=rc, in_=tot)
                    recips[t] = rc
                    nc.vector.tensor_scalar_mul(e, e, rc)  # normalize in place
                    nc.gpsimd.dma_start(
                        out=attn_o[t * 128 : (t + 1) * 128, :], in_=e
                    )

            def ctx_part(t):
                with nc.named_scope(f"ctx{t}"):
                    ett, rc = ETs[t], recips[t]
                    cx = cx_p.tile([128, D], BF16, tag="cx")
                    pss = [psum.tile([128, 512], F32, tag="ps", name=f"ps_c{t}_{er}") for er in range(2)]
                    for jc in range(NJC):
                        for er in range(2):
                            nc.tensor.matmul(
                                pss[er],
                                lhsT=ett[:, jc, :],
                                rhs=V[:, jc, er * 512 : (er + 1) * 512],
                                start=(jc == 0),
                                stop=(jc == NJC - 1),
                            )
                    for er in range(2):
                        nc.vector.tensor_scalar_mul(
                            cx[:, er * 512 : (er + 1) * 512], pss[er], rc
                        )
                    CXs[t] = cx

            def out_part(t):
                cxt = CXTs[t]
                with nc.named_scope(f"out{t}"):
                    ob = ob_p.tile([128, D], F32, tag="ob")
                    pss = [psum.tile([128, 512], F32, tag="ps", name=f"ps_o{t}_{er}") for er in range(2)]
                    for ec in range(NDC):
                        for er in range(2):
                            nc.tensor.matmul(
                                pss[er],
                                lhsT=cxt[:, ec, :],
                                rhs=wo_sb[:, ec, er * 512 : (er + 1) * 512],
                                start=(ec == 0),
                                stop=False,
                            )
                    for er in range(2):
                        nc.tensor.matmul(
                            pss[er],
                            lhsT=ones_col,
                            rhs=bor[:, er * 512 : (er + 1) * 512],
                            start=False,
                            stop=True,
                        )
                        nc.vector.tensor_copy(
                            out=ob[:, er * 512 : (er + 1) * 512], in_=pss[er]
                        )
                    nc.gpsimd.dma_start(
                        out=out_o[t * 128 : (t + 1) * 128, :], in_=ob
                    )

            # prefetch two score tiles so their softmax overlaps the V proj
            scores(0)
            scores(1)

            # ---- V projection: V[j, e] (uses wQ weights + bias, per ref) ----
            with nc.named_scope("vproj"):
                for jg in range(4):
                    chunks = load_chunks(xv, jg * 512, 512)
                    pss = {(jl, er): psum.tile([128, 512], F32, tag="ps",
                                               name=f"ps_v{jg}_{jl}_{er}")
                           for jl in range(4) for er in range(2)}
                    for c in range(NDC):
                        for jl in range(4):
                            for er in range(2):
                                nc.tensor.matmul(
                                    pss[jl, er],
                                    lhsT=chunks[c][:, jl * 128 : (jl + 1) * 128],
                                    rhs=wq_sb[:, c, er * 512 : (er + 1) * 512],
                                    start=(c == 0),
                                    stop=False,
                                )
                    for jl in range(4):
                        jt = jg * 4 + jl
                        for er in range(2):
                            nc.tensor.matmul(
                                pss[jl, er],
                                lhsT=ones_col,
                                rhs=bqr[:, er * 512 : (er + 1) * 512],
                                start=False,
                                stop=True,
                            )
                            nc.vector.tensor_scalar_max(
                                V[:, jt, er * 512 : (er + 1) * 512], pss[jl, er], 0.0
                            )

            with nc.named_scope("wo_load"):
                wo_sb = wpool.tile([128, NDC, D], BF16, tag="w")
                for c in range(NDC):
                    nc.scalar.dma_start(
                        out=wo_sb[:, c, :], in_=woT[c * 128 : (c + 1) * 128, :]
                    )

            transpose_eb(0)
            softmax_tail(0)
            for t in range(NT - 2):
                ctx_part(t)
                scores(t + 2)
                transpose_eb(t + 1)
                transpose_cx(t)
                out_part(t)
                if t + 1 < NT - 1:
                    softmax_tail(t + 1)
            ctx_part(NT - 2)
            transpose_eb(NT - 1)
            transpose_cx(NT - 2)
            softmax_tail(NT - 1)
            out_part(NT - 2)
            ctx_part(NT - 1)
            transpose_cx(NT - 1)
            out_part(NT - 1)

    if split_waits:
        # required for walrus codegen; breaks CoreSim's race detector, so
        # sim runs use split_waits=False
        _split_excess_waits(nc)
    return nc


_NC = None


def _get_nc():
    global _NC
    if _NC is None:
        _NC = build_program()
    return _NC


def build_in_maps(inputs):
    q = np.asarray(inputs["q"])
    k = np.asarray(inputs["k"])
    v = np.asarray(inputs["v"])
    wq = np.asarray(inputs["wQ_W"]).astype(bf16)
    wk = np.asarray(inputs["wK_W"]).astype(bf16)
    wo = np.asarray(inputs["out_W"]).astype(bf16)
    bq = np.asarray(inputs["wQ_b"], dtype=np.float32)
    bk = np.asarray(inputs["wK_b"], dtype=np.float32)
    bo = np.asarray(inputs["out_b"], dtype=np.float32)

    wqT = np.ascontiguousarray(wq.T)
    wkT = np.ascontiguousarray(wk.T)
    woT = np.ascontiguousarray(wo.T)
    qb = q.astype(bf16)
    kTb = [np.ascontiguousarray(k[b].T.astype(bf16)) for b in range(B)]
    vTb = [np.ascontiguousarray(v[b].T.astype(bf16)) for b in range(B)]
    bq_col = np.ascontiguousarray(bq.reshape(NDC, 128).T)
    bk_col = np.ascontiguousarray(bk.reshape(NDC, 128).T)
    bq_row = bq.astype(bf16).reshape(1, D)
    bo_row = bo.astype(bf16).reshape(1, D)

    in_maps = []
    for cid in range(N_CORES):
        b, h = divmod(cid, 2)
        in_maps.append(
            {
                "xq": np.ascontiguousarray(qb[b, h * SQ : (h + 1) * SQ, :].T),
                "xk": kTb[b],
                "xv": vTb[b],
                "wqT": wqT,
                "wkT": wkT,
                "woT": woT,
                "bq_col": bq_col,
                "bk_col": bk_col,
                "bq_row": bq_row,
                "bo_row": bo_row,
            }
        )
    return in_maps


def assemble(results):
    out = np.empty((B, S, D), np.float32)
    attn = np.empty((B, S, S), np.float32)
    for cid in range(N_CORES):
        b, h = divmod(cid, 2)
        out[b, h * SQ : (h + 1) * SQ, :] = results[cid]["out_o"]
        attn[b, h * SQ : (h + 1) * SQ, :] = results[cid]["attn_o"]
    return out, attn


def kernel(**inputs):
    nc = _get_nc()
    in_maps = build_in_maps(inputs)
    if os.environ.get("ATTEN_SPMD"):
        # one 8-way shard_map launch; fastest wall-clock but the
        # multi-device load path has hung axon terminals, so opt-in only
        res = run_bass_kernel_spmd(nc, in_maps, core_ids=list(range(N_CORES)))
        return assemble(res.results)
    # default: 8 single-device launches (same per-core program and
    # per-core HW time; the robust path on axon terminals)
    import jax
    from concourse import bass2jax

    devs = jax.devices()
    results = []
    for cid in range(N_CORES):
        with jax.default_device(devs[cid % len(devs)]):
            r = bass2jax.run_bass_via_pjrt(nc, [in_maps[cid]], n_cores=1)
        results.append(r[0])
    return assemble(results)
